# revision 1
# baseline (speedup 1.0000x reference)
"""LSTM kernel for Trainium2 (Bass/Tile), SPMD over 8 NeuronCores.

Problem: B=128, S=1024, D=256, H=512, C=10 LSTM; output = final hidden state
projected to C classes -> [B, C].

Sharding: data-parallel over batch (16 per core); weights replicated;
recurrence local per shard (no collectives).

Per-core program (two phases, one kernel launch):
  Phase 1: proj[t] = x_t @ [Wgx|Wix|Wfx|Wox ; b] for all t via full-PE GEMM
           (x stationary [128,128] tiles, W moving fp32r at 1 cyc/row),
           written to a DRAM scratch tensor.
  Phase 2: sequential recurrence. Per step: 16 matmuls (hT stationary
           [128,16], Wh moving N=512 fp32r) accumulate h@Wh into PSUM;
           DVE adds proj; ACT applies tanh/sigmoid per gate-pure 512-chunk;
           DVE cell update; PE-transposes h back into stationary hT form.
"""

import numpy as np

S, B, D, H, C = 1024, 128, 256, 512, 10
NCORES = 8
BC = B // NCORES          # batch per core
G4 = 4 * H                # fused gate width
NK_H = H // 128           # 4 K-tiles for h
NK_X = D // 128 + 1       # 2 K-tiles for x + 1 bias tile
CHUNK = 512               # PSUM-bank-sized gate chunk (one gate each: G,I,F,O)
NJ = G4 // CHUNK          # 4 chunks


def _build_nc(s_total: int):
    import concourse.bass as bass
    import concourse.mybir as mybir
    import concourse.tile as tile
    from concourse import bacc
    from concourse.masks import make_identity

    f32 = mybir.dt.float32
    f32r = mybir.dt.float32r
    AF = mybir.ActivationFunctionType
    OP = mybir.AluOpType

    m_tiles = s_total * BC // 128     # phase-1 M tiles (8 steps each)
    r_blocks = s_total // 4           # phase-2 proj DMA blocks

    nc = bacc.Bacc(
        "TRN2",
        target_bir_lowering=False,
        debug=False,
        enable_asserts=False,
        num_devices=NCORES,
    )

    xT_d = nc.dram_tensor("xT", [m_tiles, 128, NK_X, 128], f32r, kind="ExternalInput").ap()
    Wx_d = nc.dram_tensor("Wx", [NK_X, 128, G4], f32r, kind="ExternalInput").ap()
    Wh_d = nc.dram_tensor("Wh", [NK_H, 128, G4], f32r, kind="ExternalInput").ap()
    Wp_d = nc.dram_tensor("Wp", [NK_H, 128, C], f32r, kind="ExternalInput").ap()
    bp_d = nc.dram_tensor("bp", [BC, C], f32, kind="ExternalInput").ap()
    h0_d = nc.dram_tensor("h0", [128, NK_H * BC], f32r, kind="ExternalInput").ap()
    out_d = nc.dram_tensor("out", [BC, C], f32, kind="ExternalOutput").ap()

    with tile.TileContext(nc) as tc:
        with (
            tc.tile_pool(name="const", bufs=1) as const,
            tc.tile_pool(name="state", bufs=1) as state,
            tc.tile_pool(name="dram", bufs=1, space="DRAM") as dram,
        ):
            Wx_sb = const.tile([128, NK_X * G4], f32r)
            nc.sync.dma_start(
                Wx_sb[:].rearrange("p (k g) -> p k g", k=NK_X),
                Wx_d.rearrange("k p g -> p k g"),
            )
            Wh_sb = const.tile([128, NK_H * G4], f32r)
            nc.sync.dma_start(
                Wh_sb[:].rearrange("p (k g) -> p k g", k=NK_H),
                Wh_d.rearrange("k p g -> p k g"),
            )
            Wp_sb = const.tile([128, NK_H * C], f32r)
            nc.sync.dma_start(
                Wp_sb[:].rearrange("p (k c) -> p k c", k=NK_H),
                Wp_d.rearrange("k p c -> p k c"),
            )
            bp_sb = const.tile([BC, C], f32)
            nc.sync.dma_start(bp_sb[:], bp_d[:])
            ident = const.tile([BC, BC], f32)
            make_identity(nc, ident[:])

            # Recurrent state, ping-pong. hT is the transposed hidden state
            # [H-row, batch] packed as 4 K-tiles side by side: hT[:, 16k:16k+16].
            hT = [state.tile([128, NK_H * BC], f32r, tag=f"hT{i}", name=f"hT{i}") for i in range(2)]
            cs = [state.tile([BC, H], f32, tag=f"c{i}", name=f"c{i}") for i in range(2)]
            nc.sync.dma_start(hT[0][:], h0_d[:])
            nc.gpsimd.memset(cs[0][:], 0.0)

            # proj row index = 16*t + b (t = timestep, b = local batch)
            proj = dram.tile([s_total * BC, G4], f32)

            # ---------------- Phase 1: input projections ----------------
            with (
                tc.tile_pool(name="p1x", bufs=3) as p1x,
                tc.tile_pool(name="p1ps", bufs=2, space="PSUM") as p1ps,
                tc.tile_pool(name="p1st", bufs=3) as p1st,
            ):
                for m in range(m_tiles):
                    xt = p1x.tile([128, NK_X * 128], f32r)
                    nc.sync.dma_start(xt[:], xT_d[m].rearrange("p k c -> p (k c)"))
                    ps = p1ps.tile([128, G4], f32)
                    for j in range(NJ):
                        for k in range(NK_X):
                            nc.tensor.matmul(
                                ps[:, j * CHUNK:(j + 1) * CHUNK],
                                lhsT=xt[:, k * 128:(k + 1) * 128],
                                rhs=Wx_sb[:, k * G4 + j * CHUNK: k * G4 + (j + 1) * CHUNK],
                                start=(k == 0),
                                stop=(k == NK_X - 1),
                            )
                    st = p1st.tile([128, G4], f32)
                    for j in range(NJ):
                        src = ps[:, j * CHUNK:(j + 1) * CHUNK]
                        dst = st[:, j * CHUNK:(j + 1) * CHUNK]
                        if j % 2 == 0:
                            nc.vector.tensor_copy(dst, src)
                        else:
                            nc.scalar.copy(dst, src)
                    # m-tile covers steps 8m..8m+7 = proj blocks 2m, 2m+1;
                    # sbuf partition p = (t-8m)*16 + b matches (Blk s b) order.
                    nc.sync.dma_start(proj[128 * m:128 * (m + 1), :], st[:])

            # ---------------- Phase 2: recurrence ----------------
            with (
                tc.tile_pool(name="p2pj", bufs=2) as p2pj,
                tc.tile_pool(name="p2ps", bufs=1, space="PSUM") as p2ps,
                tc.tile_pool(name="p2tr", bufs=2, space="PSUM") as p2tr,
                tc.tile_pool(name="p2g", bufs=2) as p2g,
                tc.tile_pool(name="p2t", bufs=2) as p2t,
                tc.tile_pool(name="p2o", bufs=1, space="PSUM") as p2o,
            ):
                for r in range(r_blocks):
                    pj = p2pj.tile([BC, 4 * G4], f32)
                    nc.sync.dma_start(
                        pj[:].rearrange("b (s f) -> b s f", s=4),
                        proj[64 * r:64 * (r + 1), :].rearrange("(s b) f -> b s f", s=4),
                    )
                    for sidx in range(4):
                        t = 4 * r + sidx
                        cur, nxt = t % 2, (t + 1) % 2
                        ps = p2ps.tile([BC, G4], f32)
                        for j in range(NJ):
                            for k in range(NK_H):
                                nc.tensor.matmul(
                                    ps[:, j * CHUNK:(j + 1) * CHUNK],
                                    lhsT=hT[cur][:, k * BC:(k + 1) * BC],
                                    rhs=Wh_sb[:, k * G4 + j * CHUNK: k * G4 + (j + 1) * CHUNK],
                                    start=(k == 0),
                                    stop=(k == NK_H - 1),
                                )
                        gates = []
                        for j in range(NJ):
                            pre = p2t.tile([BC, CHUNK], f32, tag="pre", name="pre")
                            nc.vector.scalar_tensor_tensor(
                                pre[:],
                                ps[:, j * CHUNK:(j + 1) * CHUNK],
                                1.0,
                                pj[:, sidx * G4 + j * CHUNK: sidx * G4 + (j + 1) * CHUNK],
                                op0=OP.mult,
                                op1=OP.add,
                            )
                            gate = p2g.tile([BC, CHUNK], f32, tag=f"gate{j}", name=f"gate{j}")
                            nc.scalar.activation(
                                gate[:], pre[:],
                                AF.Tanh if j == 0 else AF.Sigmoid,
                            )
                            gates.append(gate)
                        g_, i_, f_, o_ = gates
                        gi = p2t.tile([BC, H], f32, tag="gi", name="gi")
                        nc.vector.tensor_mul(gi[:], g_[:], i_[:])
                        cn = cs[nxt]
                        nc.vector.tensor_mul(cn[:], cs[cur][:], f_[:])
                        nc.vector.tensor_add(cn[:], cn[:], gi[:])
                        th = p2t.tile([BC, H], f32, tag="th", name="th")
                        nc.scalar.activation(th[:], cn[:], AF.Tanh)
                        hn = p2t.tile([BC, H], f32, tag="hn", name="hn")
                        nc.vector.tensor_mul(hn[:], th[:], o_[:])
                        tr = p2tr.tile([128, NK_H * BC], f32)
                        for k in range(NK_H):
                            nc.tensor.transpose(
                                tr[:, k * BC:(k + 1) * BC],
                                hn[:, k * 128:(k + 1) * 128],
                                ident[:],
                            )
                        nc.vector.tensor_copy(hT[nxt][:], tr[:])

                # Final projection: out = h_S @ Wp + bp
                fin = s_total % 2
                pso = p2o.tile([BC, C], f32)
                for k in range(NK_H):
                    nc.tensor.matmul(
                        pso[:],
                        lhsT=hT[fin][:, k * BC:(k + 1) * BC],
                        rhs=Wp_sb[:, k * C:(k + 1) * C],
                        start=(k == 0),
                        stop=(k == NK_H - 1),
                    )
                res = p2g.tile([BC, C], f32, tag="res", name="res")
                nc.vector.tensor_add(res[:], pso[:], bp_sb[:])
                nc.sync.dma_start(out_d[:], res[:])

    nc.compile()
    return nc


def _prep_core_inputs(x, Wx_all, b_all, Wh_all, Wp, bp, core, s_total):
    """Build per-core numpy input map. x: [B, S, D] full batch."""
    m_tiles = s_total * BC // 128
    b0 = core * BC
    xc = x[b0:b0 + BC, :s_total, :]                     # [BC, s, D]
    # xT_host[m, p, kx, c]: stationary tiles; col c = (t - 8m)*16 + b
    a = np.ascontiguousarray(xc.transpose(2, 1, 0))     # [D, s, BC]
    a = a.reshape(D // 128, 128, m_tiles, 8, BC)        # [kx, p, m, t8, b]
    a = a.transpose(2, 1, 0, 3, 4).reshape(m_tiles, 128, D // 128, 128)
    xT = np.zeros((m_tiles, 128, NK_X, 128), dtype=np.float32)
    xT[:, :, :D // 128, :] = a
    xT[:, 0, NK_X - 1, :] = 1.0                          # bias ones-row
    return {"xT": np.ascontiguousarray(xT)}


def _prep_shared_inputs(Wgx, Wix, Wfx, Wox, Wgh, Wih, Wfh, Woh, bg, bi, bf, bo, Wph, bp):
    Wx_all = np.concatenate([Wgx, Wix, Wfx, Wox], axis=1).astype(np.float32)  # [D, G4]
    b_all = np.concatenate([bg, bi, bf, bo]).astype(np.float32)               # [G4]
    Wh_all = np.concatenate([Wgh, Wih, Wfh, Woh], axis=1).astype(np.float32)  # [H, G4]

    Wx = np.zeros((NK_X, 128, G4), dtype=np.float32)
    Wx[:D // 128] = Wx_all.reshape(D // 128, 128, G4)
    Wx[NK_X - 1, 0, :] = b_all                           # bias row (pairs with ones-row)
    Wh = np.ascontiguousarray(Wh_all.reshape(NK_H, 128, G4))
    Wp = np.ascontiguousarray(Wph.reshape(NK_H, 128, C).astype(np.float32))
    bpr = np.broadcast_to(bp.astype(np.float32), (BC, C)).copy()
    return Wx, Wh, Wp, bpr, Wx_all, b_all, Wh_all


_NC_CACHE = {}


def _get_nc(s_total):
    if s_total not in _NC_CACHE:
        _NC_CACHE[s_total] = _build_nc(s_total)
    return _NC_CACHE[s_total]


def kernel(x, Wgx, Wix, Wfx, Wox, Wgh, Wih, Wfh, Woh, bg, bi, bf, bo, Wph, bp,
           _s_total=S, _trace=False, _trace_kwargs=None):
    from concourse import bass_utils

    x = np.asarray(x, dtype=np.float32)
    args = [np.asarray(a, dtype=np.float32) for a in
            (Wgx, Wix, Wfx, Wox, Wgh, Wih, Wfh, Woh, bg, bi, bf, bo, Wph, bp)]
    Wx, Wh, Wp, bpr, Wx_all, b_all, Wh_all = _prep_shared_inputs(*args)

    nc = _get_nc(_s_total)
    in_maps = []
    for core in range(NCORES):
        m = _prep_core_inputs(x, Wx_all, b_all, Wh_all, Wp, bpr, core, _s_total)
        m.update({"Wx": Wx, "Wh": Wh, "Wp": Wp, "bp": bpr,
                  "h0": np.zeros((128, NK_H * BC), np.float32)})
        in_maps.append(m)

    kw = {}
    if _trace:
        kw["trace"] = True
        kw.update(_trace_kwargs or {})
    res = bass_utils.run_bass_kernel_spmd(nc, in_maps, core_ids=list(range(NCORES)), **kw)
    out = np.concatenate([res.results[c]["out"] for c in range(NCORES)], axis=0)
    if _trace:
        kernel._last_results = res
    return out


def _sim_selftest(s_total=16):
    """CoreSim numerics check on one core vs numpy LSTM (no hardware)."""
    from concourse.bass_interp import CoreSim

    rng = np.random.default_rng(0)
    x = rng.standard_normal((B, s_total, D), dtype=np.float32)
    mk = lambda *s: (rng.standard_normal(s, dtype=np.float32) * 0.06)
    Wgx, Wix, Wfx, Wox = (mk(D, H) for _ in range(4))
    Wgh, Wih, Wfh, Woh = (mk(H, H) for _ in range(4))
    bg, bi, bf, bo = (rng.standard_normal(H).astype(np.float32) * 0.05 for _ in range(4))
    Wph = mk(H, C)
    bp = rng.standard_normal(C).astype(np.float32) * 0.05

    def ref_np(xc):
        sig = lambda v: 1.0 / (1.0 + np.exp(-v))
        h = np.zeros((xc.shape[0], H), np.float32)
        c = np.zeros((xc.shape[0], H), np.float32)
        for t in range(s_total):
            xt = xc[:, t, :]
            g = np.tanh(xt @ Wgx + bg + h @ Wgh)
            i = sig(xt @ Wix + bi + h @ Wih)
            f = sig(xt @ Wfx + bf + h @ Wfh)
            o = sig(xt @ Wox + bo + h @ Woh)
            c = g * i + c * f
            h = np.tanh(c) * o
        return h @ Wph + bp

    args = (Wgx, Wix, Wfx, Wox, Wgh, Wih, Wfh, Woh, bg, bi, bf, bo, Wph, bp)
    Wx, Wh, Wp, bpr, Wx_all, b_all, Wh_all = _prep_shared_inputs(*args)
    nc = _build_nc(s_total)

    core = 1
    m = _prep_core_inputs(x, Wx_all, b_all, Wh_all, Wp, bpr, core, s_total)
    m.update({"Wx": Wx, "Wh": Wh, "Wp": Wp, "bp": bpr,
              "h0": np.zeros((128, NK_H * BC), np.float32)})

    sim = CoreSim(nc)
    for k, v in m.items():
        sim.tensor(k)[:] = v
    sim.simulate(check_with_hw=False)
    got = np.array(sim.tensor("out"))
    want = ref_np(x[core * BC:(core + 1) * BC])
    err = np.abs(got - want).max() / max(np.abs(want).max(), 1e-6)
    print(f"selftest S={s_total}: rel err {err:.3e}")
    assert err < 2e-2, err
    return err


if __name__ == "__main__":
    _sim_selftest(16)



# revision 9
# speedup vs baseline: 1.1641x; 1.1641x over previous
"""LSTM kernel for Trainium2 (Bass/Tile), SPMD over 8 NeuronCores.

Problem: B=128, S=1024, D=256, H=512, C=10 LSTM; output = final hidden state
projected to C classes -> [B, C].

Sharding: data-parallel over batch (16 per core); weights replicated;
recurrence local per shard (no collectives).

Per-core design (fused single pass; x read once, proj never leaves chip):
  - x-projections computed in m-tiles of 8 timesteps (M=128 = full PE
    width) a few steps ahead of the recurrence, result cast to bf16 into
    an SBUF ring -- no DRAM proj round-trip.
  - Per step, gate preactivations live in one PSUM tile [16, 2048]
    (4 banks: g|i|f|o). Each bank's accumulation group: an identity
    matmul injects proj (+bias, folded via phase-1's ones-row) with
    start=True, then 4 h @ Wh matmuls accumulate (bf16, N=512).
  - Banks are processed f,g,i,o so ACT/DVE for early gates overlap the
    later banks' matmuls; o last (shortest post-MM path).
  - Tail: cell update in fp32 [16, 512]; then PE-transpose c and o
    quarters into [128, 64] tiles; hT = tanh(cT) * oT written as bf16
    directly in the stationary layout hT[:, 16k:16k+16] (no h transpose).
"""

import numpy as np

S, B, D, H, C = 1024, 128, 256, 512, 10
NCORES = 8
BC = B // NCORES          # batch per core (16)
G4 = 4 * H                # fused gate width (2048)
NKH = H // 128            # 4 k-tiles for h
NKX = D // 128 + 1        # 2 k-tiles for x + 1 bias(ones) tile
TPM = 8                   # timesteps per phase-1 m-tile (128/BC)
LOOKAHEAD = 2             # m-tiles of proj lookahead
# gate memory order [g,i,f,o]; processing order f,g,i,o (o last)
PROC = [2, 0, 1, 3]


def _build_nc(s_total: int):
    import concourse.bass as bass
    import concourse.mybir as mybir
    import concourse.tile as tile
    from concourse import bacc
    from concourse.masks import make_identity

    f32 = mybir.dt.float32
    bf16 = mybir.dt.bfloat16
    AF = mybir.ActivationFunctionType

    n_mt = (s_total + TPM - 1) // TPM
    assert s_total % TPM == 0

    nc = bacc.Bacc(
        "TRN2",
        target_bir_lowering=False,
        debug=False,
        enable_asserts=False,
        num_devices=NCORES,
    )

    xT_d = nc.dram_tensor("xT", [n_mt, 128, NKX * 128], bf16, kind="ExternalInput").ap()
    Wx_d = nc.dram_tensor("Wx", [128, NKX * G4], bf16, kind="ExternalInput").ap()
    Wh_d = nc.dram_tensor("Wh", [128, NKH * G4], bf16, kind="ExternalInput").ap()
    i128_d = nc.dram_tensor("ident128", [128, 128], bf16, kind="ExternalInput").ap()
    Wp_d = nc.dram_tensor("Wp", [NKH, 128, C], bf16, kind="ExternalInput").ap()
    bp_d = nc.dram_tensor("bp", [BC, C], f32, kind="ExternalInput").ap()
    out_d = nc.dram_tensor("out", [BC, C], f32, kind="ExternalOutput").ap()

    with tile.TileContext(nc) as tc:
        with (
            tc.tile_pool(name="const", bufs=1) as const,
            tc.tile_pool(name="state", bufs=1) as state,
            tc.tile_pool(name="xring", bufs=3) as xring,
            tc.tile_pool(name="pring", bufs=LOOKAHEAD + 2) as pring,
            tc.tile_pool(name="p1ps", bufs=2, space="PSUM") as p1ps,
            tc.tile_pool(name="gbank", bufs=1, space="PSUM") as gbank,
            tc.tile_pool(name="tbank", bufs=1, space="PSUM") as tbank,
            tc.tile_pool(name="work", bufs=2) as work,
        ):
            Wx_sb = const.tile([128, NKX * G4], bf16)
            nc.sync.dma_start(Wx_sb[:], Wx_d[:])
            Wh_sb = const.tile([128, NKH * G4], bf16)
            nc.sync.dma_start(Wh_sb[:], Wh_d[:])
            i128_sb = const.tile([128, 128], bf16)
            nc.sync.dma_start(i128_sb[:], i128_d[:])
            Wp_sb = const.tile([128, NKH * C], bf16)
            nc.sync.dma_start(
                Wp_sb[:].rearrange("p (k c) -> p k c", k=NKH),
                Wp_d.rearrange("k p c -> p k c"),
            )
            bp_sb = const.tile([BC, C], f32)
            nc.sync.dma_start(bp_sb[:], bp_d[:])
            ident = const.tile([128, 128], f32)
            make_identity(nc, ident[:])

            # state: hT bf16 [128, NKH*BC] (hT[:, 16k:16k+16] = h k-tile),
            # c fp32 [16, 512], both ping-pong.
            hT = [state.tile([128, NKH * BC], bf16, tag=f"hT{i}", name=f"hT{i}") for i in range(2)]
            cs = [state.tile([BC, H], f32, tag=f"c{i}", name=f"c{i}") for i in range(2)]

            xtiles, ptiles = {}, {}

            def dma_xtile(m):
                xtiles[m] = xring.tile([128, NKX * 128], bf16, tag="xt", name=f"xt{m}")
                nc.sync.dma_start(xtiles[m][:], xT_d[m])

            def phase1_chunk(m, jj):
                """One gate-chunk (512 cols) of m-tile m: 3 MMs + cast-copy."""
                if jj == 0:
                    ptiles[m] = pring.tile([128, G4], bf16, tag="proj", name=f"proj{m}")
                    if m + 1 < n_mt and m + 1 not in xtiles:
                        dma_xtile(m + 1)
                xt = xtiles[m]
                ps = p1ps.tile([128, 512], f32, tag="p1", name=f"p1_{m}_{jj}")
                for k in range(NKX):
                    nc.tensor.matmul(
                        ps[:],
                        lhsT=xt[:, k * 128:(k + 1) * 128],
                        rhs=Wx_sb[:, k * G4 + jj * 512: k * G4 + (jj + 1) * 512],
                        start=(k == 0),
                        stop=(k == NKX - 1),
                    )
                dst = ptiles[m][:, jj * 512:(jj + 1) * 512]
                if jj % 2 == 0:
                    nc.vector.tensor_copy(dst, ps[:])
                else:
                    nc.scalar.copy(dst, ps[:])

            def inject(t, bank):
                """Start each gate bank's accumulation with proj(+bias)."""
                m, tt = t // TPM, t % TPM
                for j in PROC:
                    nc.tensor.matmul(
                        bank[:, j * 512:(j + 1) * 512],
                        lhsT=i128_sb[:, BC * tt:BC * (tt + 1)],
                        rhs=ptiles[m][:, j * 512:(j + 1) * 512],
                        start=True,
                        stop=(t == 0),
                    )

            # ---------------- preamble ----------------
            for m in range(min(LOOKAHEAD + 1, n_mt)):
                dma_xtile(m)
            for m in range(min(LOOKAHEAD, n_mt)):
                for jj in range(4):
                    phase1_chunk(m, jj)

            bank = gbank.tile([BC, G4], f32)
            inject(0, bank)

            p1_queue = []  # pending (m, jj) phase-1 chunks, 1 emitted per step
            for m in range(LOOKAHEAD, n_mt):
                for jj in range(4):
                    p1_queue.append((m, jj))

            qi = 0
            for t in range(s_total):
                ping = t % 2

                # h @ Wh accumulation, bank-major (f, g, i, o)
                if t > 0:
                    hprev = hT[(t + 1) % 2]
                    for j in PROC:
                        for k in range(NKH):
                            nc.tensor.matmul(
                                bank[:, j * 512:(j + 1) * 512],
                                lhsT=hprev[:, BC * k:BC * (k + 1)],
                                rhs=Wh_sb[:, k * G4 + j * 512: k * G4 + (j + 1) * 512],
                                start=False,
                                stop=(k == NKH - 1),
                            )

                # gate activations per bank, in processing order
                gf = work.tile([BC, H], f32, tag="gf", name="gf")
                nc.scalar.activation(gf[:], bank[:, 2 * 512:3 * 512], AF.Sigmoid)
                gg = work.tile([BC, H], f32, tag="gg", name="gg")
                nc.scalar.activation(gg[:], bank[:, 0 * 512:1 * 512], AF.Tanh)
                gi_ = work.tile([BC, H], f32, tag="gi", name="gi")
                nc.scalar.activation(gi_[:], bank[:, 1 * 512:2 * 512], AF.Sigmoid)
                go = work.tile([BC, H], f32, tag="go", name="go")
                nc.scalar.activation(go[:], bank[:, 3 * 512:4 * 512], AF.Sigmoid)

                # cell update (fp32): c = g*i + c*f
                prod = work.tile([BC, H], f32, tag="prod", name="prod")
                nc.vector.tensor_mul(prod[:], gg[:], gi_[:])
                if t > 0:
                    cf = work.tile([BC, H], f32, tag="cf", name="cf")
                    nc.vector.tensor_mul(cf[:], cs[(t + 1) % 2][:], gf[:])
                    nc.vector.tensor_add(cs[ping][:], prod[:], cf[:])
                else:
                    nc.vector.tensor_copy(cs[ping][:], prod[:])

                # prefill next step's banks
                if t + 1 < s_total:
                    inject(t + 1, bank)

                # transposes: o then c quarters -> [128, BC] tiles
                oT = tbank.tile([128, NKH * BC], f32, tag="oT", name="oT")
                cT = tbank.tile([128, NKH * BC], f32, tag="cT", name="cT")
                for k in range(NKH):
                    nc.tensor.transpose(
                        oT[:, BC * k:BC * (k + 1)],
                        go[:, 128 * k:128 * (k + 1)],
                        ident[0:BC, 0:BC],
                    )
                for k in range(NKH):
                    nc.tensor.transpose(
                        cT[:, BC * k:BC * (k + 1)],
                        cs[ping][:, 128 * k:128 * (k + 1)],
                        ident[0:BC, 0:BC],
                    )
                # one phase-1 chunk per 2 steps fills the PE gap here
                if t % 2 == 0 and qi < len(p1_queue):
                    phase1_chunk(*p1_queue[qi])
                    qi += 1

                thT = work.tile([128, NKH * BC], f32, tag="thT", name="thT")
                nc.scalar.activation(thT[:], cT[:], AF.Tanh)
                nc.vector.tensor_mul(hT[ping][:], thT[:], oT[:])

            # ---------------- final projection ----------------
            fin = (s_total + 1) % 2
            pso = p1ps.tile([BC, C], f32, tag="p1", name="pso")
            for k in range(NKH):
                nc.tensor.matmul(
                    pso[:],
                    lhsT=hT[fin][:, BC * k:BC * (k + 1)],
                    rhs=Wp_sb[:, k * C:(k + 1) * C],
                    start=(k == 0),
                    stop=(k == NKH - 1),
                )
            res = work.tile([BC, C], f32, tag="res", name="res")
            nc.vector.tensor_add(res[:], pso[:], bp_sb[:])
            nc.sync.dma_start(out_d[:], res[:])

    nc.compile()
    return nc


def _prep_shared_inputs(Wgx, Wix, Wfx, Wox, Wgh, Wih, Wfh, Woh, bg, bi, bf, bo, Wph, bp):
    import ml_dtypes
    bf16 = ml_dtypes.bfloat16
    Wx_all = np.concatenate([Wgx, Wix, Wfx, Wox], axis=1).astype(np.float32)  # [D, G4]
    b_all = np.concatenate([bg, bi, bf, bo]).astype(np.float32)               # [G4]
    Wh_all = np.concatenate([Wgh, Wih, Wfh, Woh], axis=1).astype(np.float32)  # [H, G4]

    Wx = np.zeros((NKX, 128, G4), dtype=np.float32)
    Wx[:D // 128] = Wx_all.reshape(D // 128, 128, G4)
    Wx[NKX - 1, 0, :] = b_all                     # bias row (pairs with ones-row)
    Wx = np.ascontiguousarray(Wx.transpose(1, 0, 2)).reshape(128, NKX * G4)
    Wh = np.ascontiguousarray(Wh_all.reshape(NKH, 128, G4).transpose(1, 0, 2)).reshape(128, NKH * G4)
    Wp = np.ascontiguousarray(Wph.reshape(NKH, 128, C))
    bpr = np.broadcast_to(bp.astype(np.float32), (BC, C)).copy()
    return (Wx.astype(bf16), Wh.astype(bf16), np.eye(128, dtype=np.float32).astype(bf16),
            Wp.astype(bf16), bpr)


def _prep_core_inputs(x, core, s_total):
    """xT[m, p, k*128 + col], col = tt*16 + b: stationary x tiles + ones row."""
    import ml_dtypes
    n_mt = s_total // TPM
    b0 = core * BC
    xc = np.asarray(x[b0:b0 + BC, :s_total, :], dtype=np.float32)   # [BC, s, D]
    a = np.ascontiguousarray(xc.transpose(2, 1, 0))                 # [D, s, BC]
    a = a.reshape(D // 128, 128, n_mt, TPM, BC)                     # [k, p, m, tt, b]
    a = a.transpose(2, 1, 0, 3, 4).reshape(n_mt, 128, D // 128, TPM * BC)
    xT = np.zeros((n_mt, 128, NKX, 128), dtype=np.float32)
    xT[:, :, :D // 128, :] = a
    xT[:, 0, NKX - 1, :] = 1.0                                      # ones row
    return {"xT": np.ascontiguousarray(xT).reshape(n_mt, 128, NKX * 128).astype(ml_dtypes.bfloat16)}


_NC_CACHE = {}


def _get_nc(s_total):
    if s_total not in _NC_CACHE:
        _NC_CACHE[s_total] = _build_nc(s_total)
    return _NC_CACHE[s_total]


def kernel(x, Wgx, Wix, Wfx, Wox, Wgh, Wih, Wfh, Woh, bg, bi, bf, bo, Wph, bp,
           _s_total=S, _trace=False, _trace_kwargs=None):
    from concourse import bass_utils

    x = np.asarray(x, dtype=np.float32)
    args = [np.asarray(a, dtype=np.float32) for a in
            (Wgx, Wix, Wfx, Wox, Wgh, Wih, Wfh, Woh, bg, bi, bf, bo, Wph, bp)]
    Wx, Wh, i16, Wp, bpr = _prep_shared_inputs(*args)

    nc = _get_nc(_s_total)
    in_maps = []
    for core in range(NCORES):
        m = _prep_core_inputs(x, core, _s_total)
        m.update({"Wx": Wx, "Wh": Wh, "ident128": i16, "Wp": Wp, "bp": bpr})
        in_maps.append(m)

    kw = {}
    if _trace:
        kw["trace"] = True
        kw.update(_trace_kwargs or {})
    res = bass_utils.run_bass_kernel_spmd(nc, in_maps, core_ids=list(range(NCORES)), **kw)
    out = np.concatenate([res.results[c]["out"] for c in range(NCORES)], axis=0)
    if _trace:
        kernel._last_results = res
    return out


def _sim_selftest(s_total=32, core=1):
    """CoreSim numerics check on one core vs numpy LSTM (no hardware)."""
    from concourse.bass_interp import CoreSim

    rng = np.random.default_rng(0)
    x = rng.standard_normal((B, s_total, D), dtype=np.float32)
    mk = lambda *s: (rng.standard_normal(s, dtype=np.float32) * 0.06)
    Wgx, Wix, Wfx, Wox = (mk(D, H) for _ in range(4))
    Wgh, Wih, Wfh, Woh = (mk(H, H) for _ in range(4))
    bg, bi, bf, bo = (rng.standard_normal(H).astype(np.float32) * 0.05 for _ in range(4))
    Wph = mk(H, C)
    bp = rng.standard_normal(C).astype(np.float32) * 0.05

    def ref_np(xc):
        sig = lambda v: 1.0 / (1.0 + np.exp(-v))
        h = np.zeros((xc.shape[0], H), np.float32)
        c = np.zeros((xc.shape[0], H), np.float32)
        for t in range(s_total):
            xt = xc[:, t, :]
            g = np.tanh(xt @ Wgx + bg + h @ Wgh)
            i = sig(xt @ Wix + bi + h @ Wih)
            f = sig(xt @ Wfx + bf + h @ Wfh)
            o = sig(xt @ Wox + bo + h @ Woh)
            c = g * i + c * f
            h = np.tanh(c) * o
        return h @ Wph + bp

    args = (Wgx, Wix, Wfx, Wox, Wgh, Wih, Wfh, Woh, bg, bi, bf, bo, Wph, bp)
    Wx, Wh, i16, Wp, bpr = _prep_shared_inputs(*args)
    nc = _build_nc(s_total)

    m = _prep_core_inputs(x, core, s_total)
    m.update({"Wx": Wx, "Wh": Wh, "ident128": i16, "Wp": Wp, "bp": bpr})

    sim = CoreSim(nc)
    for k, v in m.items():
        sim.tensor(k)[:] = v
    sim.simulate(check_with_hw=False)
    got = np.array(sim.tensor("out"))
    want = ref_np(x[core * BC:(core + 1) * BC])
    err = np.abs(got - want).max() / max(np.abs(want).max(), 1e-6)
    print(f"selftest S={s_total}: rel err {err:.3e}")
    assert err < 2e-2, err
    return err


if __name__ == "__main__":
    _sim_selftest(32)


# revision 13
# speedup vs baseline: 2.1919x; 1.8830x over previous
"""LSTM kernel for Trainium2 (Bass/Tile), SPMD over 8 NeuronCores.

Problem: B=128, S=1024, D=256, H=512, C=10 LSTM; output = final hidden state
projected to C classes -> [B, C].

Sharding: data-parallel over batch (16 per core); weights replicated;
recurrence local per shard (no collectives).

Per-core design (fused single pass; x read once, proj never leaves chip):
  - x-projections computed in m-tiles of 8 timesteps (M=128 = full PE
    width) a few steps ahead of the recurrence, result cast to bf16 into
    an SBUF ring -- no DRAM proj round-trip.
  - Per step, gate preactivations live in one PSUM tile [16, 2048]
    (4 banks: g|i|f|o). Each bank's accumulation group: an identity
    matmul injects proj (+bias, folded via phase-1's ones-row) with
    start=True, then 4 h @ Wh matmuls accumulate (bf16, N=512).
  - Banks are processed f,g,i,o so ACT/DVE for early gates overlap the
    later banks' matmuls; o last (shortest post-MM path).
  - Tail: cell update in fp32 [16, 512]; then PE-transpose c and o
    quarters into [128, 64] tiles; hT = tanh(cT) * oT written as bf16
    directly in the stationary layout hT[:, 16k:16k+16] (no h transpose).
"""

import numpy as np

S, B, D, H, C = 1024, 128, 256, 512, 10
NCORES = 8
BC = B // NCORES          # batch per core (16)
G4 = 4 * H                # fused gate width (2048)
NKH = H // 128            # 4 k-tiles for h
NKX = D // 128 + 1        # 2 k-tiles for x + 1 bias(ones) tile
TPM = 8                   # timesteps per phase-1 m-tile (128/BC)
LOOKAHEAD = 2             # m-tiles of proj lookahead
# gate memory order [g,i,f,o]; processing order f,g,i,o (o last)
PROC = [2, 0, 1, 3]


def _build_nc(s_total: int):
    import concourse.bass as bass
    import concourse.mybir as mybir
    import concourse.tile as tile
    from concourse import bacc
    from concourse.masks import make_identity

    f32 = mybir.dt.float32
    bf16 = mybir.dt.bfloat16
    AF = mybir.ActivationFunctionType

    n_mt = (s_total + TPM - 1) // TPM
    assert s_total % TPM == 0

    nc = bacc.Bacc(
        "TRN2",
        target_bir_lowering=False,
        debug=False,
        enable_asserts=False,
        num_devices=NCORES,
    )

    xT_d = nc.dram_tensor("xT", [n_mt, 128, NKX * 128], bf16, kind="ExternalInput").ap()
    Wx_d = nc.dram_tensor("Wx", [128, NKX * G4], bf16, kind="ExternalInput").ap()
    Wh_d = nc.dram_tensor("Wh", [128, NKH * G4], bf16, kind="ExternalInput").ap()
    i128_d = nc.dram_tensor("ident128", [128, 128], bf16, kind="ExternalInput").ap()
    Wp_d = nc.dram_tensor("Wp", [NKH, 128, C], bf16, kind="ExternalInput").ap()
    bp_d = nc.dram_tensor("bp", [BC, C], f32, kind="ExternalInput").ap()
    out_d = nc.dram_tensor("out", [BC, C], f32, kind="ExternalOutput").ap()

    with tile.TileContext(nc) as tc:
        with (
            tc.tile_pool(name="const", bufs=1) as const,
            tc.tile_pool(name="state", bufs=1) as state,
            tc.tile_pool(name="xring", bufs=3) as xring,
            tc.tile_pool(name="pring", bufs=LOOKAHEAD + 2) as pring,
            tc.tile_pool(name="p1ps", bufs=1, space="PSUM") as p1ps,
            tc.tile_pool(name="gbank", bufs=1, space="PSUM") as gbank,
            tc.tile_pool(name="tbank", bufs=1, space="PSUM") as tbank,
            tc.tile_pool(name="work", bufs=2) as work,
        ):
            Wx_sb = const.tile([128, NKX * G4], bf16)
            nc.sync.dma_start(Wx_sb[:], Wx_d[:])
            Wh_sb = const.tile([128, NKH * G4], bf16)
            nc.sync.dma_start(Wh_sb[:], Wh_d[:])
            i128_sb = const.tile([128, 128], bf16)
            nc.sync.dma_start(i128_sb[:], i128_d[:])
            Wp_sb = const.tile([128, NKH * C], bf16)
            nc.sync.dma_start(
                Wp_sb[:].rearrange("p (k c) -> p k c", k=NKH),
                Wp_d.rearrange("k p c -> p k c"),
            )
            bp_sb = const.tile([BC, C], f32)
            nc.sync.dma_start(bp_sb[:], bp_d[:])
            ident = const.tile([128, 128], f32)
            make_identity(nc, ident[:])

            # state split in H-halves (A = k0,k1 / B = k2,k3) so the tail
            # pipelines with the next step's matmul waves. hT_X bf16
            # [128, 2*BC] (k-tile at [:, 16k':16k'+16]), c halves [16, 256].
            hTA = [state.tile([128, 2 * BC], bf16, tag=f"hTA{i}", name=f"hTA{i}") for i in range(2)]
            hTB = [state.tile([128, 2 * BC], bf16, tag=f"hTB{i}", name=f"hTB{i}") for i in range(2)]
            csA = [state.tile([BC, H // 2], f32, tag=f"cA{i}", name=f"cA{i}") for i in range(2)]
            csB = [state.tile([BC, H // 2], f32, tag=f"cB{i}", name=f"cB{i}") for i in range(2)]

            xtiles, ptiles = {}, {}

            def dma_xtile(m):
                xtiles[m] = xring.tile([128, NKX * 128], bf16, tag="xt", name=f"xt{m}")
                nc.sync.dma_start(xtiles[m][:], xT_d[m])

            def phase1_chunk(m, jj):
                """One gate-chunk (512 cols) of m-tile m: 3 MMs + cast-copy."""
                if jj == 0:
                    ptiles[m] = pring.tile([128, G4], bf16, tag="proj", name=f"proj{m}")
                    if m + 1 < n_mt and m + 1 not in xtiles:
                        dma_xtile(m + 1)
                xt = xtiles[m]
                ps = p1ps.tile([128, 512], f32, tag="p1", name=f"p1_{m}_{jj}")
                for k in range(NKX):
                    nc.tensor.matmul(
                        ps[:],
                        lhsT=xt[:, k * 128:(k + 1) * 128],
                        rhs=Wx_sb[:, k * G4 + jj * 512: k * G4 + (jj + 1) * 512],
                        start=(k == 0),
                        stop=(k == NKX - 1),
                    )
                dst = ptiles[m][:, jj * 512:(jj + 1) * 512]
                if jj % 2 == 0:
                    nc.vector.tensor_copy(dst, ps[:])
                else:
                    nc.scalar.copy(dst, ps[:])

            def inject(t, banks):
                """Start each gate bank's accumulation with proj(+bias)."""
                m, tt = t // TPM, t % TPM
                for j in PROC:
                    nc.tensor.matmul(
                        banks[j][:],
                        lhsT=i128_sb[:, BC * tt:BC * (tt + 1)],
                        rhs=ptiles[m][:, j * 512:(j + 1) * 512],
                        start=True,
                        stop=(t == 0),
                    )

            # ---------------- preamble ----------------
            for m in range(min(LOOKAHEAD + 1, n_mt)):
                dma_xtile(m)
            for m in range(min(LOOKAHEAD, n_mt)):
                for jj in range(4):
                    phase1_chunk(m, jj)

            banks = [gbank.tile([BC, 512], f32, tag=f"gb{j}", name=f"gb{j}") for j in range(4)]
            inject(0, banks)

            p1_queue = []  # pending (m, jj) phase-1 chunks, 1 emitted per step
            for m in range(LOOKAHEAD, n_mt):
                for jj in range(4):
                    p1_queue.append((m, jj))

            qi = 0
            for t in range(s_total):
                ping = t % 2

                # h @ Wh accumulation in two k-waves: A (k0,k1) only needs
                # hT_A = first H-half of h(t-1); B (k2,k3) needs hT_B.
                if t > 0:
                    hpA = hTA[(t + 1) % 2]
                    hpB = hTB[(t + 1) % 2]
                    for j in PROC:
                        for k in (0, 1):
                            nc.tensor.matmul(
                                banks[j][:],
                                lhsT=hpA[:, BC * k:BC * (k + 1)],
                                rhs=Wh_sb[:, k * G4 + j * 512: k * G4 + (j + 1) * 512],
                                start=False,
                                stop=False,
                            )
                    for j in PROC:
                        for k in (2, 3):
                            nc.tensor.matmul(
                                banks[j][:],
                                lhsT=hpB[:, BC * (k - 2):BC * (k - 1)],
                                rhs=Wh_sb[:, k * G4 + j * 512: k * G4 + (j + 1) * 512],
                                start=False,
                                stop=(k == NKH - 1),
                            )

                # gate activations per bank, in processing order
                gf = work.tile([BC, H], f32, tag="gf", name="gf")
                nc.scalar.activation(gf[:], banks[2][:], AF.Sigmoid)
                gg = work.tile([BC, H], f32, tag="gg", name="gg")
                nc.scalar.activation(gg[:], banks[0][:], AF.Tanh)
                gi_ = work.tile([BC, H], f32, tag="gi", name="gi")
                nc.scalar.activation(gi_[:], banks[1][:], AF.Sigmoid)
                go = work.tile([BC, H], f32, tag="go", name="go")
                nc.scalar.activation(go[:], banks[3][:], AF.Sigmoid)

                # cell update (fp32, H-halved): c = g*i + c*f
                HH = H // 2
                if t > 0:
                    cfA = work.tile([BC, HH], f32, tag="cfA", name="cfA")
                    nc.vector.tensor_mul(cfA[:], csA[(t + 1) % 2][:], gf[:, 0:HH])
                    cfB = work.tile([BC, HH], f32, tag="cfB", name="cfB")
                    nc.vector.tensor_mul(cfB[:], csB[(t + 1) % 2][:], gf[:, HH:H])
                    prodA = work.tile([BC, HH], f32, tag="prodA", name="prodA")
                    nc.vector.tensor_mul(prodA[:], gg[:, 0:HH], gi_[:, 0:HH])
                    nc.vector.tensor_add(csA[ping][:], prodA[:], cfA[:])
                    prodB = work.tile([BC, HH], f32, tag="prodB", name="prodB")
                    nc.vector.tensor_mul(prodB[:], gg[:, HH:H], gi_[:, HH:H])
                    nc.vector.tensor_add(csB[ping][:], prodB[:], cfB[:])
                else:
                    nc.vector.tensor_mul(csA[ping][:], gg[:, 0:HH], gi_[:, 0:HH])
                    nc.vector.tensor_mul(csB[ping][:], gg[:, HH:H], gi_[:, HH:H])

                # prefill next step's banks (reuse same 4 PSUM tiles)
                if t + 1 < s_total:
                    banks = [gbank.tile([BC, 512], f32, tag=f"gb{j}", name=f"gb{j}_{t + 1}") for j in range(4)]
                    inject(t + 1, banks)

                # transposes: o, then c halves -> [128, BC] k-tiles
                oT = tbank.tile([128, 512], f32, tag="oT", name="oT")
                cTA = tbank.tile([128, 512], f32, tag="cTA", name="cTA")
                cTB = tbank.tile([128, 512], f32, tag="cTB", name="cTB")
                for k in range(NKH):
                    nc.tensor.transpose(
                        oT[:, BC * k:BC * (k + 1)],
                        go[:, 128 * k:128 * (k + 1)],
                        ident[0:BC, 0:BC],
                    )
                for k in (0, 1):
                    nc.tensor.transpose(
                        cTA[:, BC * k:BC * (k + 1)],
                        csA[ping][:, 128 * k:128 * (k + 1)],
                        ident[0:BC, 0:BC],
                    )
                thTA = work.tile([128, 2 * BC], f32, tag="thTA", name="thTA")
                nc.scalar.activation(thTA[:], cTA[:, 0:2 * BC], AF.Tanh)
                nc.vector.tensor_mul(hTA[ping][:], thTA[:], oT[:, 0:2 * BC])
                for k in (0, 1):
                    nc.tensor.transpose(
                        cTB[:, BC * k:BC * (k + 1)],
                        csB[ping][:, 128 * k:128 * (k + 1)],
                        ident[0:BC, 0:BC],
                    )
                # one phase-1 chunk per 2 steps fills the PE gap here
                if t % 2 == 0 and qi < len(p1_queue):
                    phase1_chunk(*p1_queue[qi])
                    qi += 1

                thTB = work.tile([128, 2 * BC], f32, tag="thTB", name="thTB")
                nc.scalar.activation(thTB[:], cTB[:, 0:2 * BC], AF.Tanh)
                nc.vector.tensor_mul(hTB[ping][:], thTB[:], oT[:, 2 * BC:4 * BC])

            # ---------------- final projection ----------------
            fin = (s_total + 1) % 2
            pso = p1ps.tile([BC, C], f32, tag="p1", name="pso")
            for k in range(NKH):
                hfin = hTA[fin] if k < 2 else hTB[fin]
                nc.tensor.matmul(
                    pso[:],
                    lhsT=hfin[:, BC * (k % 2):BC * (k % 2 + 1)],
                    rhs=Wp_sb[:, k * C:(k + 1) * C],
                    start=(k == 0),
                    stop=(k == NKH - 1),
                )
            res = work.tile([BC, C], f32, tag="res", name="res")
            nc.vector.tensor_add(res[:], pso[:], bp_sb[:])
            nc.sync.dma_start(out_d[:], res[:])

    nc.compile()
    return nc


def _prep_shared_inputs(Wgx, Wix, Wfx, Wox, Wgh, Wih, Wfh, Woh, bg, bi, bf, bo, Wph, bp):
    import ml_dtypes
    bf16 = ml_dtypes.bfloat16
    Wx_all = np.concatenate([Wgx, Wix, Wfx, Wox], axis=1).astype(np.float32)  # [D, G4]
    b_all = np.concatenate([bg, bi, bf, bo]).astype(np.float32)               # [G4]
    Wh_all = np.concatenate([Wgh, Wih, Wfh, Woh], axis=1).astype(np.float32)  # [H, G4]

    Wx = np.zeros((NKX, 128, G4), dtype=np.float32)
    Wx[:D // 128] = Wx_all.reshape(D // 128, 128, G4)
    Wx[NKX - 1, 0, :] = b_all                     # bias row (pairs with ones-row)
    Wx = np.ascontiguousarray(Wx.transpose(1, 0, 2)).reshape(128, NKX * G4)
    Wh = np.ascontiguousarray(Wh_all.reshape(NKH, 128, G4).transpose(1, 0, 2)).reshape(128, NKH * G4)
    Wp = np.ascontiguousarray(Wph.reshape(NKH, 128, C))
    bpr = np.broadcast_to(bp.astype(np.float32), (BC, C)).copy()
    return (Wx.astype(bf16), Wh.astype(bf16), np.eye(128, dtype=np.float32).astype(bf16),
            Wp.astype(bf16), bpr)


def _prep_core_inputs(x, core, s_total):
    """xT[m, p, k*128 + col], col = tt*16 + b: stationary x tiles + ones row."""
    import ml_dtypes
    n_mt = s_total // TPM
    b0 = core * BC
    xc = np.asarray(x[b0:b0 + BC, :s_total, :], dtype=np.float32)   # [BC, s, D]
    a = np.ascontiguousarray(xc.transpose(2, 1, 0))                 # [D, s, BC]
    a = a.reshape(D // 128, 128, n_mt, TPM, BC)                     # [k, p, m, tt, b]
    a = a.transpose(2, 1, 0, 3, 4).reshape(n_mt, 128, D // 128, TPM * BC)
    xT = np.zeros((n_mt, 128, NKX, 128), dtype=np.float32)
    xT[:, :, :D // 128, :] = a
    xT[:, 0, NKX - 1, :] = 1.0                                      # ones row
    return {"xT": np.ascontiguousarray(xT).reshape(n_mt, 128, NKX * 128).astype(ml_dtypes.bfloat16)}


_NC_CACHE = {}


def _get_nc(s_total):
    if s_total not in _NC_CACHE:
        _NC_CACHE[s_total] = _build_nc(s_total)
    return _NC_CACHE[s_total]


def kernel(x, Wgx, Wix, Wfx, Wox, Wgh, Wih, Wfh, Woh, bg, bi, bf, bo, Wph, bp,
           _s_total=S, _trace=False, _trace_kwargs=None):
    from concourse import bass_utils

    x = np.asarray(x, dtype=np.float32)
    args = [np.asarray(a, dtype=np.float32) for a in
            (Wgx, Wix, Wfx, Wox, Wgh, Wih, Wfh, Woh, bg, bi, bf, bo, Wph, bp)]
    Wx, Wh, i16, Wp, bpr = _prep_shared_inputs(*args)

    nc = _get_nc(_s_total)
    in_maps = []
    for core in range(NCORES):
        m = _prep_core_inputs(x, core, _s_total)
        m.update({"Wx": Wx, "Wh": Wh, "ident128": i16, "Wp": Wp, "bp": bpr})
        in_maps.append(m)

    kw = {}
    if _trace:
        kw["trace"] = True
        kw.update(_trace_kwargs or {})
    res = bass_utils.run_bass_kernel_spmd(nc, in_maps, core_ids=list(range(NCORES)), **kw)
    out = np.concatenate([res.results[c]["out"] for c in range(NCORES)], axis=0)
    if _trace:
        kernel._last_results = res
    return out


def _sim_selftest(s_total=32, core=1):
    """CoreSim numerics check on one core vs numpy LSTM (no hardware)."""
    from concourse.bass_interp import CoreSim

    rng = np.random.default_rng(0)
    x = rng.standard_normal((B, s_total, D), dtype=np.float32)
    mk = lambda *s: (rng.standard_normal(s, dtype=np.float32) * 0.06)
    Wgx, Wix, Wfx, Wox = (mk(D, H) for _ in range(4))
    Wgh, Wih, Wfh, Woh = (mk(H, H) for _ in range(4))
    bg, bi, bf, bo = (rng.standard_normal(H).astype(np.float32) * 0.05 for _ in range(4))
    Wph = mk(H, C)
    bp = rng.standard_normal(C).astype(np.float32) * 0.05

    def ref_np(xc):
        sig = lambda v: 1.0 / (1.0 + np.exp(-v))
        h = np.zeros((xc.shape[0], H), np.float32)
        c = np.zeros((xc.shape[0], H), np.float32)
        for t in range(s_total):
            xt = xc[:, t, :]
            g = np.tanh(xt @ Wgx + bg + h @ Wgh)
            i = sig(xt @ Wix + bi + h @ Wih)
            f = sig(xt @ Wfx + bf + h @ Wfh)
            o = sig(xt @ Wox + bo + h @ Woh)
            c = g * i + c * f
            h = np.tanh(c) * o
        return h @ Wph + bp

    args = (Wgx, Wix, Wfx, Wox, Wgh, Wih, Wfh, Woh, bg, bi, bf, bo, Wph, bp)
    Wx, Wh, i16, Wp, bpr = _prep_shared_inputs(*args)
    nc = _build_nc(s_total)

    m = _prep_core_inputs(x, core, s_total)
    m.update({"Wx": Wx, "Wh": Wh, "ident128": i16, "Wp": Wp, "bp": bpr})

    sim = CoreSim(nc)
    for k, v in m.items():
        sim.tensor(k)[:] = v
    sim.simulate(check_with_hw=False)
    got = np.array(sim.tensor("out"))
    want = ref_np(x[core * BC:(core + 1) * BC])
    err = np.abs(got - want).max() / max(np.abs(want).max(), 1e-6)
    print(f"selftest S={s_total}: rel err {err:.3e}")
    assert err < 2e-2, err
    return err


if __name__ == "__main__":
    _sim_selftest(32)
